# revision 11
# baseline (speedup 1.0000x reference)
"""Trainium2 Bass kernel for nn_Attention_13030930776064 (sparse_attention).

Computation (per batch row b):
    x1 = einput @ W_enc + b_enc            [S, A]
    x2 = dinput @ W_dec + b_dec            [A]
    h  = relu(x1 + x2)                     [S, A]
    scores = h @ W_att[:, 0] (+ b_att, irrelevant under softmax)
    attn = softmax(scores)                 [S]
    weights = attn @ einput                [E]
    return (weights, attn)

Strategy: pure data-parallel over B across 8 NeuronCores (4 rows/core, softmax
over S is core-local, no collectives). Host pre-transposes einput to [E, S] and
casts matmul operands to bf16 (accumulation in f32 PSUM). The big einsum runs
on the TensorEngine; relu+bias is fused into the PSUM->SBUF evacuation on
ScalarE; softmax exp runs chunk-wise (scores are O(+-5) so no max subtraction)
directly off the scores PSUM with the partial sums accumulated by the same
ACTIVATE; the attn-weighted sum over S runs chunk-wise on VectorE via
scalar_tensor_tensor against the transposed einput tiles already resident in
SBUF, with the 1/Z normalization folded into the final per-e-tile reduction.
DMA: eint streams on the sync HWDGE ring; W_enc/W_dec column pairs alternate
between the scalar HWDGE ring and the gpsimd SWDGE path so the startup weight
stream isn't serialized behind one ring's per-transfer setup cost.
"""

import numpy as np
import ml_dtypes
from contextlib import ExitStack

N_CORES = 8
B, S, E, A, D = 32, 2048, 1024, 1024, 1024
B_LOC = B // N_CORES  # 4
P = 128

_CACHE = {}
LAST_RESULT = None  # BassKernelResults of the most recent run (for profiling)


def _build(B_LOC=B_LOC, S=S, E=E, A=A, D=D, num_devices=N_CORES, debug=False):
    import concourse.bass as bass
    import concourse.tile as tile
    from concourse import bacc, mybir
    from concourse.bass import ds, ts

    ET, AT, DT = E // P, A // P, D // P
    SC = min(512, S)
    NSC = S // SC

    fp32 = mybir.dt.float32
    bf16 = mybir.dt.bfloat16
    AF = mybir.ActivationFunctionType
    ALU = mybir.AluOpType

    nc = bacc.Bacc("TRN2", target_bir_lowering=False, debug=debug, num_devices=num_devices)

    # DRAM parameters (per-core shard layouts, prepared host-side)
    eint = nc.dram_tensor("eint", [B_LOC, E, S], bf16, kind="ExternalInput").ap()
    # wcomb[p, a_t, 0, e_t, a_in] = W_enc[e_t*128+p, a_t*128+a_in]
    # wcomb[p, a_t, 1, d_t, a_in] = W_dec[d_t*128+p, a_t*128+a_in]
    wcomb = nc.dram_tensor("wcomb", [P, AT, 2, ET, P], bf16, kind="ExternalInput").ap()
    # dint[p, t, b] = dinput_shard[b, t*128 + p]
    dint = nc.dram_tensor("dint", [P, DT, B_LOC], bf16, kind="ExternalInput").ap()
    # biases[p, t] = b_enc[t*128+p] + b_dec[t*128+p]
    biases = nc.dram_tensor("biases", [P, AT], fp32, kind="ExternalInput").ap()
    # watt[p, t] = W_att[t*128+p, 0]
    watt = nc.dram_tensor("watt", [P, AT], bf16, kind="ExternalInput").ap()
    # outputs
    weightsT = nc.dram_tensor("weightsT", [P, B_LOC, ET], fp32, kind="ExternalOutput").ap()
    attn_out = nc.dram_tensor("attn", [B_LOC, S], fp32, kind="ExternalOutput").ap()

    with tile.TileContext(nc) as tc, ExitStack() as ctx:
        const = ctx.enter_context(tc.tile_pool(name="const", bufs=1))
        eint_pool = ctx.enter_context(tc.tile_pool(name="eint", bufs=3))
        hpool = ctx.enter_context(tc.tile_pool(name="h", bufs=4))
        spool = ctx.enter_context(tc.tile_pool(name="soft", bufs=2))
        bcpool = ctx.enter_context(tc.tile_pool(name="bc", bufs=2))
        junkpool = ctx.enter_context(tc.tile_pool(name="junk", bufs=2))
        wpool = ctx.enter_context(tc.tile_pool(name="wacc", bufs=2))
        psum_x1 = ctx.enter_context(tc.tile_pool(name="px1", bufs=3, space="PSUM"))
        psum_sc = ctx.enter_context(tc.tile_pool(name="psc", bufs=2, space="PSUM"))
        psum_x2 = ctx.enter_context(tc.tile_pool(name="px2", bufs=2, space="PSUM"))

        # ---- constants into SBUF
        # weight column pairs: even pairs on scalar HWDGE, odd on gpsimd SWDGE
        wcomb_sb = const.tile([P, AT, 2, ET, P], bf16)
        for a_t in range(AT):
            eng = nc.scalar if a_t % 2 == 0 else nc.gpsimd
            eng.dma_start(wcomb_sb[:, a_t], wcomb[:, a_t])
        dint_sb = const.tile([P, DT, B_LOC], bf16)
        bias_sb = const.tile([P, AT], fp32)
        watt_sb = const.tile([P, AT], bf16)

        # per-b scores chunk plans: b=0 starts with small chunks (smaller first
        # DMA/weight dependency at startup), last b ends with small chunks
        # (shorter exposed epilogue after the final PE work)
        def chunk_plan(b):
            if S < 2048:
                return [(i * SC, SC) for i in range(NSC)]
            if b == 0:
                return [(0, 256), (256, 256), (512, 512), (1024, 512), (1536, 512)]
            if b == B_LOC - 1:
                return [(0, 512), (512, 512), (1024, 512), (1536, 256), (1792, 256)]
            return [(i * SC, SC) for i in range(NSC)]

        # eint chunk DMAs for b=0 on the sync ring (pieces match b0's chunks);
        # the tiny consts ride the sync ring right after the first piece
        eint_sbs = []
        eint_sb0 = eint_pool.tile([P, ET, S], bf16, tag="eint")
        eint0_r = eint[0].rearrange("(t p) s -> p t s", p=P)
        plan0 = chunk_plan(0)
        s0_, sz_ = plan0[0]
        nc.sync.dma_start(eint_sb0[:, :, ds(s0_, sz_)], eint0_r[:, :, ds(s0_, sz_)])
        nc.sync.dma_start(dint_sb[:], dint)
        nc.sync.dma_start(bias_sb[:], biases)
        nc.sync.dma_start(watt_sb[:], watt)
        for s0_, sz_ in plan0[1:]:
            nc.sync.dma_start(eint_sb0[:, :, ds(s0_, sz_)], eint0_r[:, :, ds(s0_, sz_)])
        eint_sbs.append(eint_sb0)

        x2sb = const.tile([P, AT, B_LOC], fp32)

        def emit_x2_group(a_t):
            # x2[:, a_t, :] = dinput @ W_dec (+ b_enc + b_dec), interleaved into
            # b=0's first pass so PE doesn't stall on late weight columns
            px2 = psum_x2.tile([P, B_LOC], fp32, tag="px2")
            for d_t in range(DT):
                nc.tensor.matmul(
                    px2[:],
                    wcomb_sb[:, a_t, 1, d_t, :],
                    dint_sb[:, d_t, :],
                    start=(d_t == 0),
                    stop=(d_t == DT - 1),
                )
            nc.scalar.activation(
                x2sb[:, a_t, :], px2[:], AF.Identity, bias=bias_sb[:, ts(a_t, 1)]
            )

        for b in range(B_LOC):
            eint_sb = eint_sbs[b]
            if b + 1 < B_LOC:
                # prefetch next row's eint chunks
                nxt = eint_pool.tile([P, ET, S], bf16, tag="eint")
                nxt_r = eint[b + 1].rearrange("(t p) s -> p t s", p=P)
                for s_c in range(NSC):
                    nc.sync.dma_start(nxt[:, :, ts(s_c, SC)], nxt_r[:, :, ts(s_c, SC)])
                eint_sbs.append(nxt)

            plan = chunk_plan(b)
            NCH = len(plan)
            p_bf = spool.tile([1, S], bf16, tag="pbf")  # exp(scores), unnormalized
            pz = spool.tile([1, NCH], fp32, tag="pz")  # per-chunk sum(exp)
            attn_bc = bcpool.tile([P, S], bf16, tag="bc")
            wacc_part = wpool.tile([P, ET, NCH], fp32, tag="wpart")
            wacc = wpool.tile([P, ET], fp32, tag="wacc")

            for c_i, (s0, sz) in enumerate(plan):
                psc = psum_sc.tile([1, sz], fp32, tag="psc")
                for a_t in range(AT):
                    if b == 0 and c_i == 0:
                        emit_x2_group(a_t)
                    px1 = psum_x1.tile([P, sz], fp32, tag="px1")
                    for e_t in range(ET):
                        nc.tensor.matmul(
                            px1[:],
                            wcomb_sb[:, a_t, 0, e_t, :],
                            eint_sb[:, e_t, ds(s0, sz)],
                            start=(e_t == 0),
                            stop=(e_t == ET - 1),
                        )
                    h_sb = hpool.tile([P, sz], bf16, tag="h")
                    nc.scalar.activation(
                        h_sb[:], px1[:], AF.Relu, bias=x2sb[:, a_t, ts(b, 1)]
                    )
                    nc.tensor.matmul(
                        psc[:],
                        watt_sb[:, ts(a_t, 1)],
                        h_sb[:],
                        start=(a_t == 0),
                        stop=(a_t == AT - 1),
                    )
                # exp straight off the scores psum (bf16 out); chunk sum via accum
                nc.scalar.activation(
                    p_bf[:, ds(s0, sz)], psc[:], AF.Exp,
                    bias=0.0, accum_out=pz[:, ts(c_i, 1)],
                )
                nc.gpsimd.partition_broadcast(
                    attn_bc[:, ds(s0, sz)], p_bf[:, ds(s0, sz)]
                )
                junk = junkpool.tile([P, SC], bf16, tag="junk")
                for e_t in range(ET):
                    nc.vector.scalar_tensor_tensor(
                        out=junk[:, 0:sz],
                        in0=eint_sb[:, e_t, ds(s0, sz)],
                        scalar=1.0,
                        in1=attn_bc[:, ds(s0, sz)],
                        op0=ALU.mult,
                        op1=ALU.mult,
                        accum_out=wacc_part[:, e_t, ts(c_i, 1)],
                    )

            # normalization: z = sum(pz); rz = 1/z broadcast to all partitions
            z = spool.tile([1, 1], fp32, tag="z")
            nc.vector.tensor_reduce(
                z[:], pz[:], axis=mybir.AxisListType.X, op=ALU.add
            )
            rz = spool.tile([1, 1], fp32, tag="rz")
            nc.vector.reciprocal(rz[:], z[:])
            rz_bc = spool.tile([P, 1], fp32, tag="rzbc")
            nc.gpsimd.partition_broadcast(rz_bc[:], rz[:])

            # attn output = p * rz
            attn_f32 = spool.tile([1, S], fp32, tag="attnf")
            nc.scalar.activation(attn_f32[:], p_bf[:], AF.Identity, scale=rz[:])
            nc.sync.dma_start(attn_out[ds(b, 1)], attn_f32[:])

            # weights: scale chunk partials by rz, reduce over chunks
            junk2 = junkpool.tile([P, ET, NCH], fp32, tag="junk2")
            nc.vector.tensor_scalar(
                out=junk2[:],
                in0=wacc_part[:],
                scalar1=rz_bc[:],
                scalar2=None,
                op0=ALU.mult,
            )
            nc.vector.tensor_reduce(
                wacc[:], junk2[:], axis=mybir.AxisListType.X, op=ALU.add
            )
            nc.scalar.dma_start(weightsT[:, b], wacc[:])

    nc.compile()
    return nc


def _get_nc():
    if "nc" not in _CACHE:
        _CACHE["nc"] = _build()
    return _CACHE["nc"]


def _col_major(W, n_in_t, n_out_t):
    # [In, Out] -> [p, out_t, in_t, out_in]
    return np.ascontiguousarray(
        W.reshape(n_in_t, P, n_out_t, P).transpose(1, 2, 0, 3)
    )


def kernel(einput, dinput, W_enc, b_enc, W_dec, b_dec, W_att, b_att):
    global LAST_RESULT
    from concourse.bass_utils import run_bass_kernel_spmd

    nc = _get_nc()
    bf = ml_dtypes.bfloat16
    ET, AT, DT = E // P, A // P, D // P

    einput = np.asarray(einput, dtype=np.float32)
    dinput = np.asarray(dinput, dtype=np.float32)
    wenc2 = _col_major(np.asarray(W_enc, dtype=np.float32).astype(bf), ET, AT)
    wdec2 = _col_major(np.asarray(W_dec, dtype=np.float32).astype(bf), DT, AT)
    # [P, AT, 2, ET, P]
    wcomb = np.ascontiguousarray(np.stack([wenc2, wdec2], axis=2))
    biases = np.ascontiguousarray(
        (np.asarray(b_enc) + np.asarray(b_dec)).astype(np.float32).reshape(AT, P).T
    )
    watt = np.ascontiguousarray(np.asarray(W_att)[:, 0].reshape(AT, P).T).astype(bf)

    in_maps = []
    for c in range(N_CORES):
        sl = slice(c * B_LOC, (c + 1) * B_LOC)
        einT = np.ascontiguousarray(einput[sl].transpose(0, 2, 1)).astype(bf)
        dint = np.ascontiguousarray(
            dinput[sl].T.reshape(DT, P, B_LOC).transpose(1, 0, 2)
        ).astype(bf)
        in_maps.append(
            {
                "eint": einT,
                "wcomb": wcomb,
                "dint": dint,
                "biases": biases,
                "watt": watt,
            }
        )

    res = run_bass_kernel_spmd(nc, in_maps, core_ids=list(range(N_CORES)))
    LAST_RESULT = res

    weights = np.empty((B, E), np.float32)
    attn = np.empty((B, S), np.float32)
    for c in range(N_CORES):
        sl = slice(c * B_LOC, (c + 1) * B_LOC)
        wT = np.asarray(res.results[c]["weightsT"])  # [P, B_LOC, ET]
        weights[sl] = wT.transpose(1, 2, 0).reshape(B_LOC, E)
        attn[sl] = np.asarray(res.results[c]["attn"])
    return (weights, attn)


# revision 14
# speedup vs baseline: 1.0105x; 1.0105x over previous
"""Trainium2 Bass kernel for nn_Attention_13030930776064 (sparse_attention).

Computation (per batch row b):
    x1 = einput @ W_enc + b_enc            [S, A]
    x2 = dinput @ W_dec + b_dec            [A]
    h  = relu(x1 + x2)                     [S, A]
    scores = h @ W_att[:, 0] (+ b_att, irrelevant under softmax)
    attn = softmax(scores)                 [S]
    weights = attn @ einput                [E]
    return (weights, attn)

Strategy: pure data-parallel over B across 8 NeuronCores (4 rows/core, softmax
over S is core-local, no collectives). Host pre-transposes einput to [E, S] and
casts matmul operands to bf16 (accumulation in f32 PSUM). The big einsum runs
on the TensorEngine; relu+bias is fused into the PSUM->SBUF evacuation on
ScalarE; softmax exp runs chunk-wise (scores are O(+-5) so no max subtraction)
directly off the scores PSUM with the partial sums accumulated by the same
ACTIVATE; the attn-weighted sum over S runs chunk-wise on VectorE via
scalar_tensor_tensor against the transposed einput tiles already resident in
SBUF, with the 1/Z normalization folded into the final per-e-tile reduction.
DMA: eint streams on the sync HWDGE ring; W_enc/W_dec column pairs alternate
between the scalar HWDGE ring and the gpsimd SWDGE path so the startup weight
stream isn't serialized behind one ring's per-transfer setup cost.
"""

import numpy as np
import ml_dtypes
from contextlib import ExitStack

N_CORES = 8
B, S, E, A, D = 32, 2048, 1024, 1024, 1024
B_LOC = B // N_CORES  # 4
P = 128

_CACHE = {}
LAST_RESULT = None  # BassKernelResults of the most recent run (for profiling)


def _build(B_LOC=B_LOC, S=S, E=E, A=A, D=D, num_devices=N_CORES, debug=False):
    import concourse.bass as bass
    import concourse.tile as tile
    from concourse import bacc, mybir
    from concourse.bass import ds, ts

    ET, AT, DT = E // P, A // P, D // P
    SC = min(512, S)
    NSC = S // SC

    fp32 = mybir.dt.float32
    bf16 = mybir.dt.bfloat16
    AF = mybir.ActivationFunctionType
    ALU = mybir.AluOpType

    nc = bacc.Bacc("TRN2", target_bir_lowering=False, debug=debug, num_devices=num_devices)

    # DRAM parameters (per-core shard layouts, prepared host-side)
    eint = nc.dram_tensor("eint", [B_LOC, E, S], bf16, kind="ExternalInput").ap()
    # wcomb[p, a_t, 0, e_t, a_in] = W_enc[e_t*128+p, a_t*128+a_in]
    # wcomb[p, a_t, 1, d_t, a_in] = W_dec[d_t*128+p, a_t*128+a_in]
    wcomb = nc.dram_tensor("wcomb", [P, AT, 2, ET, P], bf16, kind="ExternalInput").ap()
    # dint[p, t, b] = dinput_shard[b, t*128 + p]
    dint = nc.dram_tensor("dint", [P, DT, B_LOC], bf16, kind="ExternalInput").ap()
    # biases[p, t] = b_enc[t*128+p] + b_dec[t*128+p]
    biases = nc.dram_tensor("biases", [P, AT], fp32, kind="ExternalInput").ap()
    # watt[p, t] = W_att[t*128+p, 0]
    watt = nc.dram_tensor("watt", [P, AT], bf16, kind="ExternalInput").ap()
    # outputs
    weightsT = nc.dram_tensor("weightsT", [P, B_LOC, ET], fp32, kind="ExternalOutput").ap()
    attn_out = nc.dram_tensor("attn", [B_LOC, S], fp32, kind="ExternalOutput").ap()

    with tile.TileContext(nc) as tc, ExitStack() as ctx:
        const = ctx.enter_context(tc.tile_pool(name="const", bufs=1))
        eint_pool = ctx.enter_context(tc.tile_pool(name="eint", bufs=3))
        hpool = ctx.enter_context(tc.tile_pool(name="h", bufs=4))
        spool = ctx.enter_context(tc.tile_pool(name="soft", bufs=2))
        bcpool = ctx.enter_context(tc.tile_pool(name="bc", bufs=2))
        junkpool = ctx.enter_context(tc.tile_pool(name="junk", bufs=2))
        wpool = ctx.enter_context(tc.tile_pool(name="wacc", bufs=2))
        psum_x1 = ctx.enter_context(tc.tile_pool(name="px1", bufs=3, space="PSUM"))
        psum_sc = ctx.enter_context(tc.tile_pool(name="psc", bufs=2, space="PSUM"))
        psum_x2 = ctx.enter_context(tc.tile_pool(name="px2", bufs=2, space="PSUM"))

        # ---- PE warmup: dummy matmuls with no input deps keep the PE busy
        # during the startup DMA window so the HAM clock-gate is at 8/8 when
        # the real matmuls start (and the PE isn't idle-throttled meanwhile).
        warmpool = ctx.enter_context(tc.tile_pool(name="warm", bufs=1))
        psum_warm = ctx.enter_context(tc.tile_pool(name="pwarm", bufs=1, space="PSUM"))
        warm_sb = warmpool.tile([P, 512], bf16)
        nc.gpsimd.memset(warm_sb[:], 0.0)
        pwarm = psum_warm.tile([P, 512], fp32)
        for _ in range(60):
            nc.tensor.matmul(
                pwarm[:], warm_sb[:, 0:P], warm_sb[:], start=True, stop=True
            )

        # ---- constants into SBUF
        # weight column pairs alternate between the two HWDGE rings
        wcomb_sb = const.tile([P, AT, 2, ET, P], bf16)
        dint_sb = const.tile([P, DT, B_LOC], bf16)
        bias_sb = const.tile([P, AT], fp32)
        watt_sb = const.tile([P, AT], bf16)

        # per-b scores chunk plans: last b ends with small chunks (shorter
        # exposed epilogue after the final PE work)
        def chunk_plan(b):
            if S < 2048:
                return [(i * SC, SC) for i in range(NSC)]
            if b == B_LOC - 1:
                return [(0, 512), (512, 512), (1024, 512), (1536, 256), (1792, 256)]
            return [(i * SC, SC) for i in range(NSC)]

        # Startup DMA order: the first eint chunk is split e_t-wise across the
        # two HWDGE rings, the tiny consts ride behind it on sync, then the
        # weight column pairs alternate rings; remaining eint pieces + all
        # prefetches follow on sync.
        eint_sbs = []
        eint_sb0 = eint_pool.tile([P, ET, S], bf16, tag="eint")
        eint0_r = eint[0].rearrange("(t p) s -> p t s", p=P)
        nc.scalar.dma_start(eint_sb0[:, 0 : ET // 2, ds(0, SC)], eint0_r[:, 0 : ET // 2, ds(0, SC)])
        nc.sync.dma_start(eint_sb0[:, ET // 2 : ET, ds(0, SC)], eint0_r[:, ET // 2 : ET, ds(0, SC)])
        nc.sync.dma_start(dint_sb[:], dint)
        nc.sync.dma_start(bias_sb[:], biases)
        nc.sync.dma_start(watt_sb[:], watt)
        for a_t in range(AT):
            eng = nc.scalar if a_t % 2 == 0 else nc.sync
            eng.dma_start(wcomb_sb[:, a_t], wcomb[:, a_t])
        for s0_, sz_ in chunk_plan(0)[1:]:
            nc.sync.dma_start(eint_sb0[:, :, ds(s0_, sz_)], eint0_r[:, :, ds(s0_, sz_)])
        eint_sbs.append(eint_sb0)

        x2sb = const.tile([P, AT, B_LOC], fp32)

        def emit_x2_group(a_t):
            # x2[:, a_t, :] = dinput @ W_dec (+ b_enc + b_dec), interleaved into
            # b=0's first pass so PE doesn't stall on late weight columns
            px2 = psum_x2.tile([P, B_LOC], fp32, tag="px2")
            for d_t in range(DT):
                nc.tensor.matmul(
                    px2[:],
                    wcomb_sb[:, a_t, 1, d_t, :],
                    dint_sb[:, d_t, :],
                    start=(d_t == 0),
                    stop=(d_t == DT - 1),
                )
            nc.scalar.activation(
                x2sb[:, a_t, :], px2[:], AF.Identity, bias=bias_sb[:, ts(a_t, 1)]
            )

        for b in range(B_LOC):
            eint_sb = eint_sbs[b]
            if b + 1 < B_LOC:
                # prefetch next row's eint chunks
                nxt = eint_pool.tile([P, ET, S], bf16, tag="eint")
                nxt_r = eint[b + 1].rearrange("(t p) s -> p t s", p=P)
                for s_c in range(NSC):
                    nc.sync.dma_start(nxt[:, :, ts(s_c, SC)], nxt_r[:, :, ts(s_c, SC)])
                eint_sbs.append(nxt)

            plan = chunk_plan(b)
            NCH = len(plan)
            p_bf = spool.tile([1, S], bf16, tag="pbf")  # exp(scores), unnormalized
            pz = spool.tile([1, NCH], fp32, tag="pz")  # per-chunk sum(exp)
            attn_bc = bcpool.tile([P, S], bf16, tag="bc")
            wacc_part = wpool.tile([P, ET, NCH], fp32, tag="wpart")
            wacc = wpool.tile([P, ET], fp32, tag="wacc")

            for c_i, (s0, sz) in enumerate(plan):
                psc = psum_sc.tile([1, sz], fp32, tag="psc")
                for a_t in range(AT):
                    if b == 0 and c_i == 0:
                        emit_x2_group(a_t)
                    px1 = psum_x1.tile([P, sz], fp32, tag="px1")
                    for e_t in range(ET):
                        nc.tensor.matmul(
                            px1[:],
                            wcomb_sb[:, a_t, 0, e_t, :],
                            eint_sb[:, e_t, ds(s0, sz)],
                            start=(e_t == 0),
                            stop=(e_t == ET - 1),
                        )
                    h_sb = hpool.tile([P, sz], bf16, tag="h")
                    nc.scalar.activation(
                        h_sb[:], px1[:], AF.Relu, bias=x2sb[:, a_t, ts(b, 1)]
                    )
                    nc.tensor.matmul(
                        psc[:],
                        watt_sb[:, ts(a_t, 1)],
                        h_sb[:],
                        start=(a_t == 0),
                        stop=(a_t == AT - 1),
                    )
                # exp straight off the scores psum (bf16 out); chunk sum via accum
                nc.scalar.activation(
                    p_bf[:, ds(s0, sz)], psc[:], AF.Exp,
                    bias=0.0, accum_out=pz[:, ts(c_i, 1)],
                )
                nc.gpsimd.partition_broadcast(
                    attn_bc[:, ds(s0, sz)], p_bf[:, ds(s0, sz)]
                )
                junk = junkpool.tile([P, SC], bf16, tag="junk")
                for e_t in range(ET):
                    nc.vector.scalar_tensor_tensor(
                        out=junk[:, 0:sz],
                        in0=eint_sb[:, e_t, ds(s0, sz)],
                        scalar=1.0,
                        in1=attn_bc[:, ds(s0, sz)],
                        op0=ALU.mult,
                        op1=ALU.mult,
                        accum_out=wacc_part[:, e_t, ts(c_i, 1)],
                    )

            # normalization: z = sum(pz); rz = 1/z broadcast to all partitions
            z = spool.tile([1, 1], fp32, tag="z")
            nc.vector.tensor_reduce(
                z[:], pz[:], axis=mybir.AxisListType.X, op=ALU.add
            )
            rz = spool.tile([1, 1], fp32, tag="rz")
            nc.vector.reciprocal(rz[:], z[:])
            rz_bc = spool.tile([P, 1], fp32, tag="rzbc")
            nc.gpsimd.partition_broadcast(rz_bc[:], rz[:])

            # attn output = p * rz
            attn_f32 = spool.tile([1, S], fp32, tag="attnf")
            nc.scalar.activation(attn_f32[:], p_bf[:], AF.Identity, scale=rz[:])
            nc.sync.dma_start(attn_out[ds(b, 1)], attn_f32[:])

            # weights: scale chunk partials by rz, reduce over chunks
            junk2 = junkpool.tile([P, ET, NCH], fp32, tag="junk2")
            nc.vector.tensor_scalar(
                out=junk2[:],
                in0=wacc_part[:],
                scalar1=rz_bc[:],
                scalar2=None,
                op0=ALU.mult,
            )
            nc.vector.tensor_reduce(
                wacc[:], junk2[:], axis=mybir.AxisListType.X, op=ALU.add
            )
            nc.scalar.dma_start(weightsT[:, b], wacc[:])

    nc.compile()
    return nc


def _get_nc():
    if "nc" not in _CACHE:
        _CACHE["nc"] = _build()
    return _CACHE["nc"]


def _col_major(W, n_in_t, n_out_t):
    # [In, Out] -> [p, out_t, in_t, out_in]
    return np.ascontiguousarray(
        W.reshape(n_in_t, P, n_out_t, P).transpose(1, 2, 0, 3)
    )


def kernel(einput, dinput, W_enc, b_enc, W_dec, b_dec, W_att, b_att):
    global LAST_RESULT
    from concourse.bass_utils import run_bass_kernel_spmd

    nc = _get_nc()
    bf = ml_dtypes.bfloat16
    ET, AT, DT = E // P, A // P, D // P

    einput = np.asarray(einput, dtype=np.float32)
    dinput = np.asarray(dinput, dtype=np.float32)
    wenc2 = _col_major(np.asarray(W_enc, dtype=np.float32).astype(bf), ET, AT)
    wdec2 = _col_major(np.asarray(W_dec, dtype=np.float32).astype(bf), DT, AT)
    # [P, AT, 2, ET, P]
    wcomb = np.ascontiguousarray(np.stack([wenc2, wdec2], axis=2))
    biases = np.ascontiguousarray(
        (np.asarray(b_enc) + np.asarray(b_dec)).astype(np.float32).reshape(AT, P).T
    )
    watt = np.ascontiguousarray(np.asarray(W_att)[:, 0].reshape(AT, P).T).astype(bf)

    in_maps = []
    for c in range(N_CORES):
        sl = slice(c * B_LOC, (c + 1) * B_LOC)
        einT = np.ascontiguousarray(einput[sl].transpose(0, 2, 1)).astype(bf)
        dint = np.ascontiguousarray(
            dinput[sl].T.reshape(DT, P, B_LOC).transpose(1, 0, 2)
        ).astype(bf)
        in_maps.append(
            {
                "eint": einT,
                "wcomb": wcomb,
                "dint": dint,
                "biases": biases,
                "watt": watt,
            }
        )

    res = run_bass_kernel_spmd(nc, in_maps, core_ids=list(range(N_CORES)))
    LAST_RESULT = res

    weights = np.empty((B, E), np.float32)
    attn = np.empty((B, S), np.float32)
    for c in range(N_CORES):
        sl = slice(c * B_LOC, (c + 1) * B_LOC)
        wT = np.asarray(res.results[c]["weightsT"])  # [P, B_LOC, ET]
        weights[sl] = wT.transpose(1, 2, 0).reshape(B_LOC, E)
        attn[sl] = np.asarray(res.results[c]["attn"])
    return (weights, attn)


# revision 17
# speedup vs baseline: 1.0204x; 1.0099x over previous
"""Trainium2 Bass kernel for nn_Attention_13030930776064 (sparse_attention).

Computation (per batch row b):
    x1 = einput @ W_enc + b_enc            [S, A]
    x2 = dinput @ W_dec + b_dec            [A]
    h  = relu(x1 + x2)                     [S, A]
    scores = h @ W_att[:, 0] (+ b_att, irrelevant under softmax)
    attn = softmax(scores)                 [S]
    weights = attn @ einput                [E]
    return (weights, attn)

Strategy: pure data-parallel over B across 8 NeuronCores (4 rows/core, softmax
over S is core-local, no collectives). Host pre-transposes einput to [E, S] and
casts matmul operands to bf16 (accumulation in f32 PSUM). The big einsum runs
on the TensorEngine; relu+bias is fused into the PSUM->SBUF evacuation on
ScalarE; softmax exp runs chunk-wise (scores are O(+-5) so no max subtraction)
directly off the scores PSUM with the partial sums accumulated by the same
ACTIVATE; the attn-weighted sum over S runs chunk-wise on VectorE via
scalar_tensor_tensor against the transposed einput tiles already resident in
SBUF, with the 1/Z normalization folded into the final per-e-tile reduction.
DMA: eint streams on the sync HWDGE ring; W_enc/W_dec column pairs alternate
between the scalar HWDGE ring and the gpsimd SWDGE path so the startup weight
stream isn't serialized behind one ring's per-transfer setup cost.
"""

import numpy as np
import ml_dtypes
from contextlib import ExitStack

N_CORES = 8
B, S, E, A, D = 32, 2048, 1024, 1024, 1024
B_LOC = B // N_CORES  # 4
P = 128

_CACHE = {}
LAST_RESULT = None  # BassKernelResults of the most recent run (for profiling)


def _build(B_LOC=B_LOC, S=S, E=E, A=A, D=D, num_devices=N_CORES, debug=False):
    import concourse.bass as bass
    import concourse.tile as tile
    from concourse import bacc, mybir
    from concourse.bass import ds, ts

    ET, AT, DT = E // P, A // P, D // P
    SC = min(512, S)
    NSC = S // SC

    fp32 = mybir.dt.float32
    bf16 = mybir.dt.bfloat16
    AF = mybir.ActivationFunctionType
    ALU = mybir.AluOpType

    nc = bacc.Bacc("TRN2", target_bir_lowering=False, debug=debug, num_devices=num_devices)

    # DRAM parameters (per-core shard layouts, prepared host-side)
    eint = nc.dram_tensor("eint", [B_LOC, E, S], bf16, kind="ExternalInput").ap()
    # wcomb[p, a_t, 0, e_t, a_in] = W_enc[e_t*128+p, a_t*128+a_in]
    # wcomb[p, a_t, 1, d_t, a_in] = W_dec[d_t*128+p, a_t*128+a_in]
    wcomb = nc.dram_tensor("wcomb", [P, AT, 2, ET, P], bf16, kind="ExternalInput").ap()
    # dint[p, t, b] = dinput_shard[b, t*128 + p]
    dint = nc.dram_tensor("dint", [P, DT, B_LOC], bf16, kind="ExternalInput").ap()
    # biases[p, t] = b_enc[t*128+p] + b_dec[t*128+p]
    biases = nc.dram_tensor("biases", [P, AT], fp32, kind="ExternalInput").ap()
    # watt[p, t] = W_att[t*128+p, 0]
    watt = nc.dram_tensor("watt", [P, AT], bf16, kind="ExternalInput").ap()
    # outputs
    weightsT = nc.dram_tensor("weightsT", [P, B_LOC, ET], fp32, kind="ExternalOutput").ap()
    attn_out = nc.dram_tensor("attn", [B_LOC, S], fp32, kind="ExternalOutput").ap()

    with tile.TileContext(nc) as tc, ExitStack() as ctx:
        const = ctx.enter_context(tc.tile_pool(name="const", bufs=1))
        eint_pool = ctx.enter_context(tc.tile_pool(name="eint", bufs=3))
        hpool = ctx.enter_context(tc.tile_pool(name="h", bufs=4))
        spool = ctx.enter_context(tc.tile_pool(name="soft", bufs=2))
        bcpool = ctx.enter_context(tc.tile_pool(name="bc", bufs=2))
        junkpool = ctx.enter_context(tc.tile_pool(name="junk", bufs=2))
        wpool = ctx.enter_context(tc.tile_pool(name="wacc", bufs=2))
        psum_x1 = ctx.enter_context(tc.tile_pool(name="px1", bufs=3, space="PSUM"))
        psum_sc = ctx.enter_context(tc.tile_pool(name="psc", bufs=2, space="PSUM"))
        psum_x2 = ctx.enter_context(tc.tile_pool(name="px2", bufs=2, space="PSUM"))

        # ---- PE warmup: dummy matmuls with no input deps keep the PE busy
        # during the startup DMA window so the HAM clock-gate is at 8/8 when
        # the real matmuls start (and the PE isn't idle-throttled meanwhile).
        warmpool = ctx.enter_context(tc.tile_pool(name="warm", bufs=1))
        psum_warm = ctx.enter_context(tc.tile_pool(name="pwarm", bufs=1, space="PSUM"))
        warm_sb = warmpool.tile([P, 512], bf16)
        nc.vector.memset(warm_sb[:], 0.0)
        pwarm = psum_warm.tile([P, 512], fp32)
        for _ in range(44):
            nc.tensor.matmul(
                pwarm[:], warm_sb[:, 0:P], warm_sb[:], start=True, stop=True
            )

        # ---- constants into SBUF
        # weight column pairs alternate between the two HWDGE rings
        wcomb_sb = const.tile([P, AT, 2, ET, P], bf16)
        dint_sb = const.tile([P, DT, B_LOC], bf16)
        bias_sb = const.tile([P, AT], fp32)
        watt_sb = const.tile([P, AT], bf16)

        # per-b scores chunk plans: last b ends with small chunks (shorter
        # exposed epilogue after the final PE work)
        def chunk_plan(b):
            if S < 2048:
                return [(i * SC, SC) for i in range(NSC)]
            if b == B_LOC - 1:
                return [(0, 512), (512, 512), (1024, 512), (1536, 256), (1792, 256)]
            return [(i * SC, SC) for i in range(NSC)]

        # Startup DMA order: the first eint chunk is split e_t-wise across the
        # two HWDGE rings, the tiny consts ride behind it on sync, then the
        # weight column pairs alternate rings; remaining eint pieces + all
        # prefetches follow on sync.
        eint_sbs = []
        eint_sb0 = eint_pool.tile([P, ET, S], bf16, tag="eint")
        eint0_r = eint[0].rearrange("(t p) s -> p t s", p=P)
        nc.scalar.dma_start(eint_sb0[:, 0 : ET // 2, ds(0, SC)], eint0_r[:, 0 : ET // 2, ds(0, SC)])
        nc.sync.dma_start(eint_sb0[:, ET // 2 : ET, ds(0, SC)], eint0_r[:, ET // 2 : ET, ds(0, SC)])
        nc.sync.dma_start(dint_sb[:], dint)
        nc.sync.dma_start(bias_sb[:], biases)
        nc.sync.dma_start(watt_sb[:], watt)
        for a_t in range(AT):
            eng = nc.scalar if a_t % 2 == 0 else nc.sync
            eng.dma_start(wcomb_sb[:, a_t], wcomb[:, a_t])
        for s0_, sz_ in chunk_plan(0)[1:]:
            nc.sync.dma_start(eint_sb0[:, :, ds(s0_, sz_)], eint0_r[:, :, ds(s0_, sz_)])
        eint_sbs.append(eint_sb0)

        x2sb = const.tile([P, AT, B_LOC], fp32)

        def emit_x2_group(a_t):
            # x2[:, a_t, :] = dinput @ W_dec (+ b_enc + b_dec), interleaved into
            # b=0's first pass so PE doesn't stall on late weight columns
            px2 = psum_x2.tile([P, B_LOC], fp32, tag="px2")
            for d_t in range(DT):
                nc.tensor.matmul(
                    px2[:],
                    wcomb_sb[:, a_t, 1, d_t, :],
                    dint_sb[:, d_t, :],
                    start=(d_t == 0),
                    stop=(d_t == DT - 1),
                )
            nc.scalar.activation(
                x2sb[:, a_t, :], px2[:], AF.Identity, bias=bias_sb[:, ts(a_t, 1)]
            )

        for b in range(B_LOC):
            eint_sb = eint_sbs[b]
            if b + 1 < B_LOC:
                # prefetch next row's eint chunks
                nxt = eint_pool.tile([P, ET, S], bf16, tag="eint")
                nxt_r = eint[b + 1].rearrange("(t p) s -> p t s", p=P)
                for s_c in range(NSC):
                    nc.sync.dma_start(nxt[:, :, ts(s_c, SC)], nxt_r[:, :, ts(s_c, SC)])
                eint_sbs.append(nxt)

            plan = chunk_plan(b)
            NCH = len(plan)
            p_bf = spool.tile([1, S], bf16, tag="pbf")  # exp(scores), unnormalized
            pz = spool.tile([1, NCH], fp32, tag="pz")  # per-chunk sum(exp)
            attn_bc = bcpool.tile([P, S], bf16, tag="bc")
            wacc_part = wpool.tile([P, ET, NCH], fp32, tag="wpart")
            wacc = wpool.tile([P, ET], fp32, tag="wacc")

            for c_i, (s0, sz) in enumerate(plan):
                psc = psum_sc.tile([1, sz], fp32, tag="psc")
                for a_t in range(AT):
                    if b == 0 and c_i == 0:
                        emit_x2_group(a_t)
                    px1 = psum_x1.tile([P, sz], fp32, tag="px1")
                    for e_t in range(ET):
                        nc.tensor.matmul(
                            px1[:],
                            wcomb_sb[:, a_t, 0, e_t, :],
                            eint_sb[:, e_t, ds(s0, sz)],
                            start=(e_t == 0),
                            stop=(e_t == ET - 1),
                        )
                    h_sb = hpool.tile([P, sz], bf16, tag="h")
                    nc.scalar.activation(
                        h_sb[:], px1[:], AF.Relu, bias=x2sb[:, a_t, ts(b, 1)]
                    )
                    nc.tensor.matmul(
                        psc[:],
                        watt_sb[:, ts(a_t, 1)],
                        h_sb[:],
                        start=(a_t == 0),
                        stop=(a_t == AT - 1),
                    )
                # exp straight off the scores psum (bf16 out); chunk sum via accum
                nc.scalar.activation(
                    p_bf[:, ds(s0, sz)], psc[:], AF.Exp,
                    bias=0.0, accum_out=pz[:, ts(c_i, 1)],
                )
                nc.gpsimd.partition_broadcast(
                    attn_bc[:, ds(s0, sz)], p_bf[:, ds(s0, sz)]
                )
                junk = junkpool.tile([P, SC], bf16, tag="junk")
                for e_t in range(ET):
                    nc.vector.scalar_tensor_tensor(
                        out=junk[:, 0:sz],
                        in0=eint_sb[:, e_t, ds(s0, sz)],
                        scalar=1.0,
                        in1=attn_bc[:, ds(s0, sz)],
                        op0=ALU.mult,
                        op1=ALU.mult,
                        accum_out=wacc_part[:, e_t, ts(c_i, 1)],
                    )

            # normalization: z = sum(pz); rz = 1/z broadcast to all partitions
            z = spool.tile([1, 1], fp32, tag="z")
            nc.vector.tensor_reduce(
                z[:], pz[:], axis=mybir.AxisListType.X, op=ALU.add
            )
            rz = spool.tile([1, 1], fp32, tag="rz")
            nc.vector.reciprocal(rz[:], z[:])
            rz_bc = spool.tile([P, 1], fp32, tag="rzbc")
            nc.gpsimd.partition_broadcast(rz_bc[:], rz[:])

            # attn output = p * rz (chunked on the last b so the DMA pipelines
            # with the scale inside the exposed epilogue)
            attn_f32 = spool.tile([1, S], fp32, tag="attnf")
            out_pieces = plan if b == B_LOC - 1 else [(0, S)]
            for s0_, sz_ in out_pieces:
                nc.scalar.activation(
                    attn_f32[:, ds(s0_, sz_)], p_bf[:, ds(s0_, sz_)],
                    AF.Identity, scale=rz[:],
                )
                nc.sync.dma_start(
                    attn_out[ds(b, 1), ds(s0_, sz_)], attn_f32[:, ds(s0_, sz_)]
                )

            # weights: scale chunk partials by rz, reduce over chunks
            junk2 = junkpool.tile([P, ET, NCH], fp32, tag="junk2")
            nc.vector.tensor_scalar(
                out=junk2[:],
                in0=wacc_part[:],
                scalar1=rz_bc[:],
                scalar2=None,
                op0=ALU.mult,
            )
            nc.vector.tensor_reduce(
                wacc[:], junk2[:], axis=mybir.AxisListType.X, op=ALU.add
            )
            nc.scalar.dma_start(weightsT[:, b], wacc[:])

    nc.compile()
    return nc


def _get_nc():
    if "nc" not in _CACHE:
        _CACHE["nc"] = _build()
    return _CACHE["nc"]


def _col_major(W, n_in_t, n_out_t):
    # [In, Out] -> [p, out_t, in_t, out_in]
    return np.ascontiguousarray(
        W.reshape(n_in_t, P, n_out_t, P).transpose(1, 2, 0, 3)
    )


def kernel(einput, dinput, W_enc, b_enc, W_dec, b_dec, W_att, b_att):
    global LAST_RESULT
    from concourse.bass_utils import run_bass_kernel_spmd

    nc = _get_nc()
    bf = ml_dtypes.bfloat16
    ET, AT, DT = E // P, A // P, D // P

    einput = np.asarray(einput, dtype=np.float32)
    dinput = np.asarray(dinput, dtype=np.float32)
    wenc2 = _col_major(np.asarray(W_enc, dtype=np.float32).astype(bf), ET, AT)
    wdec2 = _col_major(np.asarray(W_dec, dtype=np.float32).astype(bf), DT, AT)
    # [P, AT, 2, ET, P]
    wcomb = np.ascontiguousarray(np.stack([wenc2, wdec2], axis=2))
    biases = np.ascontiguousarray(
        (np.asarray(b_enc) + np.asarray(b_dec)).astype(np.float32).reshape(AT, P).T
    )
    watt = np.ascontiguousarray(np.asarray(W_att)[:, 0].reshape(AT, P).T).astype(bf)

    in_maps = []
    for c in range(N_CORES):
        sl = slice(c * B_LOC, (c + 1) * B_LOC)
        einT = np.ascontiguousarray(einput[sl].transpose(0, 2, 1)).astype(bf)
        dint = np.ascontiguousarray(
            dinput[sl].T.reshape(DT, P, B_LOC).transpose(1, 0, 2)
        ).astype(bf)
        in_maps.append(
            {
                "eint": einT,
                "wcomb": wcomb,
                "dint": dint,
                "biases": biases,
                "watt": watt,
            }
        )

    res = run_bass_kernel_spmd(nc, in_maps, core_ids=list(range(N_CORES)))
    LAST_RESULT = res

    weights = np.empty((B, E), np.float32)
    attn = np.empty((B, S), np.float32)
    for c in range(N_CORES):
        sl = slice(c * B_LOC, (c + 1) * B_LOC)
        wT = np.asarray(res.results[c]["weightsT"])  # [P, B_LOC, ET]
        weights[sl] = wT.transpose(1, 2, 0).reshape(B_LOC, E)
        attn[sl] = np.asarray(res.results[c]["attn"])
    return (weights, attn)


# revision 22
# speedup vs baseline: 1.1641x; 1.1408x over previous
"""Trainium2 Bass kernel for nn_Attention_13030930776064 (sparse_attention).

Computation (per batch row b):
    x1 = einput @ W_enc + b_enc            [S, A]
    x2 = dinput @ W_dec + b_dec            [A]
    h  = relu(x1 + x2)                     [S, A]
    scores = h @ W_att[:, 0] (+ b_att, irrelevant under softmax)
    attn = softmax(scores)                 [S]
    weights = attn @ einput                [E]
    return (weights, attn)

Strategy: pure data-parallel over B across 8 NeuronCores (4 rows/core, softmax
over S is core-local, no collectives). Host pre-transposes einput to [E, S] and
casts matmul operands to bf16 (accumulation in f32 PSUM). The big einsum runs
on the TensorEngine; relu+bias is fused into the PSUM->SBUF evacuation on
ScalarE; softmax exp runs chunk-wise (scores are O(+-5) so no max subtraction)
directly off the scores PSUM with the partial sums accumulated by the same
ACTIVATE; the attn-weighted sum over S runs chunk-wise on VectorE via
scalar_tensor_tensor against the transposed einput tiles already resident in
SBUF, with the 1/Z normalization folded into the final per-e-tile reduction.
DMA: eint streams on the sync HWDGE ring; W_enc/W_dec column pairs alternate
between the scalar HWDGE ring and the gpsimd SWDGE path so the startup weight
stream isn't serialized behind one ring's per-transfer setup cost.
"""

import numpy as np
import ml_dtypes
from contextlib import ExitStack

N_CORES = 8
B, S, E, A, D = 32, 2048, 1024, 1024, 1024
B_LOC = B // N_CORES  # 4
P = 128

_CACHE = {}
LAST_RESULT = None  # BassKernelResults of the most recent run (for profiling)


def _build(B_LOC=B_LOC, S=S, E=E, A=A, D=D, num_devices=N_CORES, debug=False):
    import concourse.bass as bass
    import concourse.tile as tile
    from concourse import bacc, mybir
    from concourse.bass import ds, ts

    ET, AT, DT = E // P, A // P, D // P
    SC = min(512, S)
    NSC = S // SC

    fp32 = mybir.dt.float32
    bf16 = mybir.dt.bfloat16
    AF = mybir.ActivationFunctionType
    ALU = mybir.AluOpType

    nc = bacc.Bacc("TRN2", target_bir_lowering=False, debug=debug, num_devices=num_devices)

    # DRAM parameters (per-core shard layouts, prepared host-side)
    eint = nc.dram_tensor("eint", [B_LOC, E, S], bf16, kind="ExternalInput").ap()
    # wcomb[p, a_t, 0, e_t, a_in] = W_enc[e_t*128+p, a_t*128+a_in]
    # wcomb[p, a_t, 1, d_t, a_in] = W_dec[d_t*128+p, a_t*128+a_in]
    wcomb = nc.dram_tensor("wcomb", [P, AT, 2, ET, P], bf16, kind="ExternalInput").ap()
    # dint[p, t, b] = dinput_shard[b, t*128 + p]
    dint = nc.dram_tensor("dint", [P, DT, B_LOC], bf16, kind="ExternalInput").ap()
    # biases[p, t] = b_enc[t*128+p] + b_dec[t*128+p]
    biases = nc.dram_tensor("biases", [P, AT], fp32, kind="ExternalInput").ap()
    # watt[p, t] = W_att[t*128+p, 0]
    watt = nc.dram_tensor("watt", [P, AT], fp32, kind="ExternalInput").ap()
    # outputs
    weightsT = nc.dram_tensor("weightsT", [P, B_LOC, ET], fp32, kind="ExternalOutput").ap()
    attn_out = nc.dram_tensor("attn", [B_LOC, S], fp32, kind="ExternalOutput").ap()

    with tile.TileContext(nc) as tc, ExitStack() as ctx:
        const = ctx.enter_context(tc.tile_pool(name="const", bufs=1))
        eint_pool = ctx.enter_context(tc.tile_pool(name="eint", bufs=3))
        hpool = ctx.enter_context(tc.tile_pool(name="h", bufs=4))
        spool = ctx.enter_context(tc.tile_pool(name="soft", bufs=2))
        bcpool = ctx.enter_context(tc.tile_pool(name="bc", bufs=2))
        junkpool = ctx.enter_context(tc.tile_pool(name="junk", bufs=2))
        wpool = ctx.enter_context(tc.tile_pool(name="wacc", bufs=2))
        psum_x1 = ctx.enter_context(tc.tile_pool(name="px1", bufs=3, space="PSUM"))
        psum_sc = ctx.enter_context(tc.tile_pool(name="psc", bufs=2, space="PSUM"))
        psum_x2 = ctx.enter_context(tc.tile_pool(name="px2", bufs=2, space="PSUM"))

        # ---- PE warmup: dummy matmuls with no input deps keep the PE busy
        # during the startup DMA window so the HAM clock-gate is at 8/8 when
        # the real matmuls start (and the PE isn't idle-throttled meanwhile).
        warmpool = ctx.enter_context(tc.tile_pool(name="warm", bufs=1))
        psum_warm = ctx.enter_context(tc.tile_pool(name="pwarm", bufs=1, space="PSUM"))
        warm_sb = warmpool.tile([P, 512], bf16)
        nc.vector.memset(warm_sb[:], 0.0)
        ones_sb = warmpool.tile([P, 1], fp32)
        nc.vector.memset(ones_sb[:], 1.0)
        pwarm = psum_warm.tile([P, 512], fp32)
        for _ in range(44):
            nc.tensor.matmul(
                pwarm[:], warm_sb[:, 0:P], warm_sb[:], start=True, stop=True
            )

        # ---- constants into SBUF
        # weight column pairs alternate between the two HWDGE rings
        wcomb_sb = const.tile([P, AT, 2, ET, P], bf16)
        dint_sb = const.tile([P, DT, B_LOC], bf16)
        bias_sb = const.tile([P, AT], fp32)
        watt_sb = const.tile([P, AT], fp32)

        # per-b scores chunk plans: last b ends with small chunks (shorter
        # exposed epilogue after the final PE work)
        def chunk_plan(b):
            if S < 2048:
                return [(i * SC, SC) for i in range(NSC)]
            if b == B_LOC - 1:
                return [(0, 512), (512, 512), (1024, 512), (1536, 256), (1792, 256)]
            return [(i * SC, SC) for i in range(NSC)]

        # Startup DMA order: the first eint chunk is split e_t-wise across the
        # two HWDGE rings, the tiny consts ride behind it on sync, then the
        # weight column pairs alternate rings; remaining eint pieces + all
        # prefetches follow on sync.
        eint_sbs = []
        eint_sb0 = eint_pool.tile([P, ET, S], bf16, tag="eint")
        eint0_r = eint[0].rearrange("(t p) s -> p t s", p=P)
        nc.scalar.dma_start(eint_sb0[:, 0 : ET // 2, ds(0, SC)], eint0_r[:, 0 : ET // 2, ds(0, SC)])
        nc.sync.dma_start(eint_sb0[:, ET // 2 : ET, ds(0, SC)], eint0_r[:, ET // 2 : ET, ds(0, SC)])
        nc.sync.dma_start(dint_sb[:], dint)
        nc.sync.dma_start(bias_sb[:], biases)
        nc.sync.dma_start(watt_sb[:], watt)
        for a_t in range(AT):
            eng = nc.scalar if a_t % 2 == 0 else nc.sync
            eng.dma_start(wcomb_sb[:, a_t], wcomb[:, a_t])
        for s0_, sz_ in chunk_plan(0)[1:]:
            nc.sync.dma_start(eint_sb0[:, :, ds(s0_, sz_)], eint0_r[:, :, ds(s0_, sz_)])
        eint_sbs.append(eint_sb0)

        x2sb = const.tile([P, AT, B_LOC], fp32)

        def emit_x2_group(a_t):
            # x2[:, a_t, :] = dinput @ W_dec (+ b_enc + b_dec), interleaved into
            # b=0's first pass so PE doesn't stall on late weight columns
            px2 = psum_x2.tile([P, B_LOC], fp32, tag="px2")
            for d_t in range(DT):
                nc.tensor.matmul(
                    px2[:],
                    wcomb_sb[:, a_t, 1, d_t, :],
                    dint_sb[:, d_t, :],
                    start=(d_t == 0),
                    stop=(d_t == DT - 1),
                )
            nc.scalar.activation(
                x2sb[:, a_t, :], px2[:], AF.Identity, bias=bias_sb[:, ts(a_t, 1)]
            )

        for b in range(B_LOC):
            eint_sb = eint_sbs[b]
            if b + 1 < B_LOC:
                # prefetch next row's eint chunks
                nxt = eint_pool.tile([P, ET, S], bf16, tag="eint")
                nxt_r = eint[b + 1].rearrange("(t p) s -> p t s", p=P)
                for s_c in range(NSC):
                    nc.sync.dma_start(nxt[:, :, ts(s_c, SC)], nxt_r[:, :, ts(s_c, SC)])
                eint_sbs.append(nxt)

            plan = chunk_plan(b)
            NCH = len(plan)
            p_bf = spool.tile([1, S], bf16, tag="pbf")  # exp(scores), unnormalized
            pz = spool.tile([1, NCH], fp32, tag="pz")  # per-chunk sum(exp)
            attn_bc = bcpool.tile([P, S], bf16, tag="bc")
            wacc_part = wpool.tile([P, ET, NCH], fp32, tag="wpart")
            wacc = wpool.tile([P, ET], fp32, tag="wacc")

            for c_i, (s0, sz) in enumerate(plan):
                psc = psum_sc.tile([1, sz], fp32, tag="psc")
                hacc = hpool.tile([P, SC], fp32, tag="hacc")
                for a_t in range(AT):
                    if b == 0 and c_i == 0:
                        emit_x2_group(a_t)
                    px1 = psum_x1.tile([P, sz], fp32, tag="px1")
                    for e_t in range(ET):
                        nc.tensor.matmul(
                            px1[:],
                            wcomb_sb[:, a_t, 0, e_t, :],
                            eint_sb[:, e_t, ds(s0, sz)],
                            start=(e_t == 0),
                            stop=(e_t == ET - 1),
                        )
                    h_sb = hpool.tile([P, sz], bf16, tag="h")
                    nc.scalar.activation(
                        h_sb[:], px1[:], AF.Relu, bias=x2sb[:, a_t, ts(b, 1)]
                    )
                    # hacc += watt[:, a_t] * h  (DVE; partition-reduced below by
                    # a single ones-vector matmul instead of 8 M=1 matmuls)
                    if a_t == 0:
                        nc.vector.tensor_scalar(
                            out=hacc[:, 0:sz], in0=h_sb[:],
                            scalar1=watt_sb[:, ts(a_t, 1)], scalar2=None,
                            op0=ALU.mult,
                        )
                    else:
                        nc.vector.scalar_tensor_tensor(
                            out=hacc[:, 0:sz], in0=h_sb[:],
                            scalar=watt_sb[:, ts(a_t, 1)], in1=hacc[:, 0:sz],
                            op0=ALU.mult, op1=ALU.add,
                        )
                nc.tensor.matmul(
                    psc[:], ones_sb[:], hacc[:, 0:sz], start=True, stop=True
                )
                # exp straight off the scores psum (bf16 out); chunk sum via accum
                nc.scalar.activation(
                    p_bf[:, ds(s0, sz)], psc[:], AF.Exp,
                    bias=0.0, accum_out=pz[:, ts(c_i, 1)],
                )
                nc.gpsimd.partition_broadcast(
                    attn_bc[:, ds(s0, sz)], p_bf[:, ds(s0, sz)]
                )
                junk = junkpool.tile([P, SC], bf16, tag="junk")
                for e_t in range(ET):
                    nc.vector.scalar_tensor_tensor(
                        out=junk[:, 0:sz],
                        in0=eint_sb[:, e_t, ds(s0, sz)],
                        scalar=1.0,
                        in1=attn_bc[:, ds(s0, sz)],
                        op0=ALU.mult,
                        op1=ALU.mult,
                        accum_out=wacc_part[:, e_t, ts(c_i, 1)],
                    )

            # normalization: z = sum(pz); rz = 1/z broadcast to all partitions
            z = spool.tile([1, 1], fp32, tag="z")
            nc.vector.tensor_reduce(
                z[:], pz[:], axis=mybir.AxisListType.X, op=ALU.add
            )
            rz = spool.tile([1, 1], fp32, tag="rz")
            nc.vector.reciprocal(rz[:], z[:])
            rz_bc = spool.tile([P, 1], fp32, tag="rzbc")
            nc.gpsimd.partition_broadcast(rz_bc[:], rz[:])

            # attn output = p * rz (chunked on the last b so the DMA pipelines
            # with the scale inside the exposed epilogue)
            attn_f32 = spool.tile([1, S], fp32, tag="attnf")
            out_pieces = plan if b == B_LOC - 1 else [(0, S)]
            for s0_, sz_ in out_pieces:
                nc.scalar.activation(
                    attn_f32[:, ds(s0_, sz_)], p_bf[:, ds(s0_, sz_)],
                    AF.Identity, scale=rz[:],
                )
                nc.sync.dma_start(
                    attn_out[ds(b, 1), ds(s0_, sz_)], attn_f32[:, ds(s0_, sz_)]
                )

            # weights: scale chunk partials by rz, reduce over chunks
            junk2 = junkpool.tile([P, ET, NCH], fp32, tag="junk2")
            nc.vector.tensor_scalar(
                out=junk2[:],
                in0=wacc_part[:],
                scalar1=rz_bc[:],
                scalar2=None,
                op0=ALU.mult,
            )
            nc.vector.tensor_reduce(
                wacc[:], junk2[:], axis=mybir.AxisListType.X, op=ALU.add
            )
            nc.scalar.dma_start(weightsT[:, b], wacc[:])

    nc.compile()
    return nc


def _get_nc():
    if "nc" not in _CACHE:
        _CACHE["nc"] = _build()
    return _CACHE["nc"]


def _col_major(W, n_in_t, n_out_t):
    # [In, Out] -> [p, out_t, in_t, out_in]
    return np.ascontiguousarray(
        W.reshape(n_in_t, P, n_out_t, P).transpose(1, 2, 0, 3)
    )


def kernel(einput, dinput, W_enc, b_enc, W_dec, b_dec, W_att, b_att):
    global LAST_RESULT
    from concourse.bass_utils import run_bass_kernel_spmd

    nc = _get_nc()
    bf = ml_dtypes.bfloat16
    ET, AT, DT = E // P, A // P, D // P

    einput = np.asarray(einput, dtype=np.float32)
    dinput = np.asarray(dinput, dtype=np.float32)
    wenc2 = _col_major(np.asarray(W_enc, dtype=np.float32).astype(bf), ET, AT)
    wdec2 = _col_major(np.asarray(W_dec, dtype=np.float32).astype(bf), DT, AT)
    # [P, AT, 2, ET, P]
    wcomb = np.ascontiguousarray(np.stack([wenc2, wdec2], axis=2))
    biases = np.ascontiguousarray(
        (np.asarray(b_enc) + np.asarray(b_dec)).astype(np.float32).reshape(AT, P).T
    )
    watt = np.ascontiguousarray(
        np.asarray(W_att, dtype=np.float32)[:, 0].reshape(AT, P).T
    )

    in_maps = []
    for c in range(N_CORES):
        sl = slice(c * B_LOC, (c + 1) * B_LOC)
        einT = np.ascontiguousarray(einput[sl].transpose(0, 2, 1)).astype(bf)
        dint = np.ascontiguousarray(
            dinput[sl].T.reshape(DT, P, B_LOC).transpose(1, 0, 2)
        ).astype(bf)
        in_maps.append(
            {
                "eint": einT,
                "wcomb": wcomb,
                "dint": dint,
                "biases": biases,
                "watt": watt,
            }
        )

    res = run_bass_kernel_spmd(nc, in_maps, core_ids=list(range(N_CORES)))
    LAST_RESULT = res

    weights = np.empty((B, E), np.float32)
    attn = np.empty((B, S), np.float32)
    for c in range(N_CORES):
        sl = slice(c * B_LOC, (c + 1) * B_LOC)
        wT = np.asarray(res.results[c]["weightsT"])  # [P, B_LOC, ET]
        weights[sl] = wT.transpose(1, 2, 0).reshape(B_LOC, E)
        attn[sl] = np.asarray(res.results[c]["attn"])
    return (weights, attn)
